# revision 10
# baseline (speedup 1.0000x reference)
"""Bass/Trainium2 kernel for masked attention + resize (nn_BaseAttender).

Full-input contract: kernel(**inputs) takes the complete unsharded tensors,
shards batch-wise across 8 NeuronCores (2 batches per core), runs one SPMD
Bass program, and gathers the full [16, 1024, 256] output.

Math (per batch):
    logits  = Q @ K^T / sqrt(512)              [1024, 2048]
    attn    = softmax(where(mask==0, -1e9, logits))
    context = attn @ V                          [1024, 512]
    out     = context @ W^T + b                 [1024, 256]

Implementation notes:
  - softmax without max-subtraction: logits are O(5) so exp() is safe in
    fp32/bf16, and `where(mask==0, -inf)` + softmax == exp(logits)*mask
    normalized by its sum (exact: masked entries contribute exactly 0).
  - all matmuls run in bf16 (PE processes 1 element/cell/cycle regardless of
    dtype; fp32 would be 4x slower) with fp32 PSUM accumulation.
  - scores are computed in [q, k] layout so the int32 mask loads naturally
    and row sums (softmax denominators) come free via accum_out.
  - exp*mask is PE-transposed to [k, q] so phase 2 (attn @ V) and phase 3
    (resize) use only natural-layout stationary/moving operands.
  - the 1/denominator scaling commutes past the k-contraction and the
    v-contraction, so it is applied once at the very end on [q, 256] tiles.
"""

import sys

sys.path.insert(0, "/opt/trn_rl_repo")

import numpy as np

import concourse.bass as bass
import concourse.tile as tile
from concourse import bacc, mybir
from concourse.bass_utils import run_bass_kernel_spmd
from concourse.masks import make_identity

# problem shape (hardcoded per contract)
B, NQ, NK, D, V, O = 16, 1024, 2048, 512, 512, 256
N_CORES = 8
B_LOC = B // N_CORES          # batches per core
SCALE = 1.0 / np.sqrt(np.float32(512.0))

P = 128
DT = D // P                   # 4 d-tiles (contraction of phase 1)
KT = NK // P                  # 16 k-tiles
QT = NQ // P                  # 8 q-tiles
KC = NK // 512                # 4 k-chunks of 512 (phase-1 moving dim)
QC = NQ // 512                # 2 q-chunks of 512 (phase-2 moving dim)
VT = V // P                   # 4 v-tiles
OT = O // P                   # 2 o-tiles

F32 = mybir.dt.float32
BF = mybir.dt.bfloat16
I32 = mybir.dt.int32

_NC_CACHE = {}


def _build(loop_n=None, no_dma=False, kq_bf16_cast=False):
    nc = bacc.Bacc(num_swdge_queues=2)
    keys = nc.declare_dram_parameter("keys", [B_LOC, NK, D], F32, isOutput=False)
    queries = nc.declare_dram_parameter("queries", [B_LOC, NQ, D], F32, isOutput=False)
    values = nc.declare_dram_parameter("values", [B_LOC, NK, V], F32, isOutput=False)
    mask = nc.declare_dram_parameter("mask", [B_LOC, NQ, NK], I32, isOutput=False)
    w_r = nc.declare_dram_parameter("w_resize", [O, V], F32, isOutput=False)
    b_r = nc.declare_dram_parameter("b_resize", [P, O], F32, isOutput=False)
    out = nc.declare_dram_parameter("out", [B_LOC, NQ, O], F32, isOutput=True)

    with tile.TileContext(nc) as tc:
        with (
            tc.tile_pool(name="const", bufs=1) as constp,
            tc.tile_pool(name="qt_sb", bufs=2) as qtp,
            tc.tile_pool(name="kt_sb", bufs=2) as ktp,
            tc.tile_pool(name="v_sb", bufs=1) as vp,
            tc.tile_pool(name="expt_sb", bufs=2) as etp,
            tc.tile_pool(name="ctxt_sb", bufs=1) as ctp,
            tc.tile_pool(name="nat", bufs=3) as natp,
            tc.tile_pool(name="natbf", bufs=3) as natbfp,          # staging tiles for transposes
            tc.tile_pool(name="maskrow", bufs=2) as mp,
            tc.tile_pool(name="expm", bufs=4) as emp,
            tc.tile_pool(name="den", bufs=8) as dnp,
            tc.tile_pool(name="outsb", bufs=2) as osp,
            tc.tile_pool(name="ps_s", bufs=2, space="PSUM") as psp,    # phase-1 scores
            tc.tile_pool(name="ps_tr", bufs=2, space="PSUM") as trp,   # transposes (bf16)
            tc.tile_pool(name="ps_c", bufs=2, space="PSUM") as pcp,    # phase-2 context
            tc.tile_pool(name="ps_o", bufs=2, space="PSUM") as pop,    # phase-3 out
        ):
            ident = constp.tile([P, P], BF)
            make_identity(nc, ident[:])
            identf = constp.tile([P, P], F32)
            make_identity(nc, identf[:])

            bias_sb = constp.tile([P, O], F32)
            nc.sync.dma_start(bias_sb[:], b_r[:])

            # ---- stage W^T: [O, V] fp32 -> wt_sb [v=128, vt, o] bf16 ----
            wt_sb = constp.tile([P, VT, O], BF)
            for ot in range(OT):
                wnat = natp.tile([P, 1, V], F32, tag="nat")
                nc.sync.dma_start(wnat[:, 0, :], w_r[ot * P:(ot + 1) * P, :])
                ps_w = trp.tile([P, 4, P], F32, tag="tr")
                for vt in range(VT):
                    nc.tensor.transpose(ps_w[:, vt, :], wnat[:, 0, vt * P:(vt + 1) * P], identf[:])
                nc.scalar.copy(wt_sb[:, :, ot * P:(ot + 1) * P], ps_w[:])

            def emit_core_body():
              for b in range(B_LOC):
                # ---- per-batch staging: Q^T, K^T (PE transposes), V (cast loads) ----
                qt_sb = qtp.tile([P, DT, NQ], BF)      # [d=128, dt, q]
                q_view = queries[b].rearrange("(a p) d -> p a d", p=P)
                for g in range(QT // 4):
                    qnat = natp.tile([P, 4, D], F32, tag="nat")
                    if not no_dma:
                        nc.sync.dma_start(qnat[:], q_view[:, 4 * g:4 * (g + 1), :])
                    if kq_bf16_cast:
                        qbf = natbfp.tile([P, 4, D], BF, tag="natbf")
                        nc.scalar.copy(qbf[:], qnat[:])
                    for j in range(4):
                        qn = 4 * g + j
                        if kq_bf16_cast:
                            ps_t = trp.tile([P, 4, P], BF, tag="tr")
                            for dt in range(DT):
                                nc.tensor.transpose(ps_t[:, dt, :], qbf[:, j, dt * P:(dt + 1) * P], ident[:])
                        else:
                            ps_t = trp.tile([P, 4, P], F32, tag="tr")
                            for dt in range(DT):
                                nc.tensor.transpose(ps_t[:, dt, :], qnat[:, j, dt * P:(dt + 1) * P], identf[:])
                        nc.scalar.copy(qt_sb[:, :, qn * P:(qn + 1) * P], ps_t[:])

                kt_sb = ktp.tile([P, DT, NK], BF)      # [d=128, dt, k]
                k_view = keys[b].rearrange("(a p) d -> p a d", p=P)
                for g in range(KT // 4):
                    knat = natp.tile([P, 4, D], F32, tag="nat")
                    if not no_dma:
                        nc.scalar.dma_start(knat[:], k_view[:, 4 * g:4 * (g + 1), :])
                    if kq_bf16_cast:
                        kbf = natbfp.tile([P, 4, D], BF, tag="natbf")
                        nc.vector.tensor_copy(kbf[:], knat[:])
                    for j in range(4):
                        kt = 4 * g + j
                        if kq_bf16_cast:
                            ps_t = trp.tile([P, 4, P], BF, tag="tr")
                            for dt in range(DT):
                                nc.tensor.transpose(ps_t[:, dt, :], kbf[:, j, dt * P:(dt + 1) * P], ident[:])
                        else:
                            ps_t = trp.tile([P, 4, P], F32, tag="tr")
                            for dt in range(DT):
                                nc.tensor.transpose(ps_t[:, dt, :], knat[:, j, dt * P:(dt + 1) * P], identf[:])
                        nc.vector.tensor_copy(kt_sb[:, :, kt * P:(kt + 1) * P], ps_t[:])

                v_sb = vp.tile([P, KT, V], BF)         # [k=128, kt, v]
                v_view = values[b].rearrange("(a p) v -> p a v", p=P)
                for g in range(KT // 4):
                    vnat = natp.tile([P, 4, V], F32, tag="nat")
                    if not no_dma:
                        nc.scalar.dma_start(vnat[:], v_view[:, 4 * g:4 * (g + 1), :])
                    nc.vector.tensor_copy(v_sb[:, 4 * g:4 * (g + 1), :], vnat[:])

                expt_sb = etp.tile([P, KT, NQ], BF)    # [k=128, kt, q]
                recips = dnp.tile([P, QT], F32, tag="recips")

                # ---- phase 1: scores [q, k], exp, mask, transpose to [k, q] ----
                for qt in range(QT):
                    mrow = mp.tile([P, NK], I32)
                    if not no_dma:
                        nc.sync.dma_start(mrow[:], mask[b, qt * P:(qt + 1) * P, :])
                    den4 = dnp.tile([P, KC], F32, tag="den4")
                    for kc in range(KC):
                        ps_s = psp.tile([P, 512], F32, tag="scores")
                        for dt in range(DT):
                            nc.tensor.matmul(
                                ps_s[:],
                                qt_sb[:, dt, qt * P:(qt + 1) * P],
                                kt_sb[:, dt, kc * 512:(kc + 1) * 512],
                                start=(dt == 0),
                                stop=(dt == DT - 1),
                            )
                        expm = emp.tile([P, 512], BF, tag="expm")
                        nc.scalar.activation(
                            expm[:], ps_s[:], mybir.ActivationFunctionType.Exp, scale=float(SCALE)
                        )
                        expmm = emp.tile([P, 512], BF, tag="expmm")
                        nc.vector.scalar_tensor_tensor(
                            expmm[:], expm[:], 1.0, mrow[:, kc * 512:(kc + 1) * 512],
                            mybir.AluOpType.bypass, mybir.AluOpType.mult,
                            accum_out=den4[:, kc:kc + 1],
                        )
                        ps_t = trp.tile([P, 4, P], BF, tag="tr")
                        for kb in range(4):
                            nc.tensor.transpose(
                                ps_t[:, kb, :], expmm[:, kb * P:(kb + 1) * P], ident[:]
                            )
                        # copy [k=128, 4 k-blocks, q=128] into expt_sb
                        cp = nc.scalar if (qt * KC + kc) % 2 == 0 else nc.vector
                        if cp is nc.scalar:
                            nc.scalar.copy(
                                expt_sb[:, kc * 4:(kc + 1) * 4, qt * P:(qt + 1) * P], ps_t[:]
                            )
                        else:
                            nc.vector.tensor_copy(
                                expt_sb[:, kc * 4:(kc + 1) * 4, qt * P:(qt + 1) * P], ps_t[:]
                            )
                    densum = dnp.tile([P, 1], F32, tag="densum")
                    nc.vector.tensor_reduce(
                        out=densum[:], in_=den4[:], axis=mybir.AxisListType.X,
                        op=mybir.AluOpType.add,
                    )
                    nc.vector.reciprocal(recips[:, qt:qt + 1], densum[:])

                # ---- phase 2: context^T [v, q] = V^T @ exp^T ----
                ctxt_sb = ctp.tile([P, VT, NQ], BF)
                for qc in range(QC):
                    for vt in range(VT):
                        ps_c = pcp.tile([P, 512], F32, tag="ctx")
                        for kt in range(KT):
                            nc.tensor.matmul(
                                ps_c[:],
                                v_sb[:, kt, vt * P:(vt + 1) * P],
                                expt_sb[:, kt, qc * 512:(qc + 1) * 512],
                                start=(kt == 0),
                                stop=(kt == KT - 1),
                            )
                        nc.scalar.copy(ctxt_sb[:, vt, qc * 512:(qc + 1) * 512], ps_c[:])

                # ---- phase 3: out [q, o] = ctx^T.T @ W^T, scaled + bias ----
                for qt in range(QT):
                    ps_o = pop.tile([P, O], F32, tag="out")
                    for vt in range(VT):
                        nc.tensor.matmul(
                            ps_o[:],
                            ctxt_sb[:, vt, qt * P:(qt + 1) * P],
                            wt_sb[:, vt, :],
                            start=(vt == 0),
                            stop=(vt == VT - 1),
                        )
                    out_sb = osp.tile([P, O], F32)
                    nc.vector.scalar_tensor_tensor(
                        out_sb[:], ps_o[:], recips[:, qt:qt + 1], bias_sb[:],
                        mybir.AluOpType.mult, mybir.AluOpType.add,
                    )
                    if not no_dma:
                        nc.sync.dma_start(out[b, qt * P:(qt + 1) * P, :], out_sb[:])

            if loop_n is None:
                emit_core_body()
            else:
                with tc.For_i(0, loop_n, 1) as _i:
                    emit_core_body()

    nc.finalize()
    return nc


def kernel(keys, queries, values, mask, W_resize, b_resize):
    keys = np.ascontiguousarray(np.asarray(keys, dtype=np.float32))
    queries = np.ascontiguousarray(np.asarray(queries, dtype=np.float32))
    values = np.ascontiguousarray(np.asarray(values, dtype=np.float32))
    mask = np.ascontiguousarray(np.asarray(mask, dtype=np.int32))
    w_r = np.ascontiguousarray(np.asarray(W_resize, dtype=np.float32))
    b_rep = np.ascontiguousarray(
        np.broadcast_to(np.asarray(b_resize, dtype=np.float32).reshape(1, O), (P, O))
    )

    if "nc" not in _NC_CACHE:
        _NC_CACHE["nc"] = _build()
    nc = _NC_CACHE["nc"]

    in_maps = []
    for c in range(N_CORES):
        s = slice(c * B_LOC, (c + 1) * B_LOC)
        in_maps.append(
            {
                "keys": keys[s],
                "queries": queries[s],
                "values": values[s],
                "mask": mask[s],
                "w_resize": w_r,
                "b_resize": b_rep,
            }
        )

    r = run_bass_kernel_spmd(nc, in_maps, list(range(N_CORES)))
    return np.concatenate([r.results[c]["out"] for c in range(N_CORES)], axis=0)
